# revision 17
# baseline (speedup 1.0000x reference)
"""Trainium2 Bass kernel for nn_GAT_609885356262 (GNN message passing).

Sharding: data-parallel over batch B=32 -> 4 samples per core on 8 cores.
All weights/adjacency replicated; each core runs the full model on its
batch slice; host gathers.

Host-side prep (mirrors the original torch model's CPU/scipy path for the
rescaled Laplacian): lmax via eigvalsh, Lr = 2L/lmax - I, mask offset
(M-1)*150, weight transposes, and q = W @ a folds (weight preprocessing).

Device-side structure per (batch, head):
  PE:  B1 = f1_i + f2_j + 150*(M-1)  (identity x maskoff + rank-2 matmuls,
       bf16, accumulated in PSUM); masked entries end at exp(0.2(e-150))~1e-13
  DVE: leaky = max(B1, 0.2*B1)  via one scalar_tensor_tensor
  ACT: P = Exp(leaky)           (bf16)
  PE:  UT = H_aug^T @ P  (H_aug carries a ones column -> row 64 = softmax sums)
  DVE: norm = relu(UT) * (1/s broadcast)  via one scalar_tensor_tensor
  DVE: pooled = windowed max over 8 nodes (tensor_reduce, axis=X)
       [relu(elu(x)) == relu(x), and relu/maxpool commute, so elu vanishes]
  PE:  fc1 streamed in bf16, interleaved with attention so DMA overlaps
Tail (dec / nn / fc2 / log_softmax) also runs on-device.
"""

import sys

for _p in ("/opt/trn_rl_repo", "/opt/pypackages"):
    if _p not in sys.path:
        sys.path.insert(0, _p)

import numpy as np
import ml_dtypes

import concourse.bass as bass
import concourse.bacc as bacc_mod
import concourse.mybir as mybir
import concourse.tile as tile
from concourse import bass_utils
from concourse import dve_ops as _dvo
from concourse.dve_spec import Spec as _Spec, Src0 as _Src0, C0 as _C0, maxx as _maxx
from concourse.dve_spec import lower as _dve_lower, _has_src1 as _dve_has_src1
from concourse.dve_uop import DveOpSpec as _DveOpSpec


def _register_leaky_op():
    """Register a custom single-pass DVE op: out = max(in0, in0*s0).

    Self-contained: appends to concourse.dve_ops registries at import time.
    The uops sha is computed locally so DveOp.compile's drift check passes.
    """
    name = "LEAKY_MAX_ANT"
    if name in _dvo._SUB_OPCODE_FOR_NAME:
        return next(op for op in _dvo.OPS if op.name == name)
    spec = _Spec(
        body=_maxx(_Src0, _Src0 * _C0),
        reference=lambda in0, in1, s0, s1, imm2: np.maximum(
            in0.astype(np.float32), in0.astype(np.float32) * s0
        ),
    )
    row = _dvo._CUSTOM_DVE_ROW_BASE + len(_dvo.OPS)
    assert row < 0x20
    shas = {}
    for ver in ("v3", "v4"):
        uops = _dve_lower(spec, ver=ver)
        s = _DveOpSpec(name=name, opcode=row, uops=uops, rd1_en=_dve_has_src1(spec))
        shas[ver] = s.sha(ver)
    op = _dvo.DveOp(name, spec, subdim=False, uops_sha=shas)
    _dvo.OPS.append(op)
    _dvo._SUB_OPCODE_FOR_NAME[name] = row
    _dvo.CUSTOM_DVE_SPECS[name] = spec
    return op


LEAKY_OP = _register_leaky_op()

BF16 = mybir.dt.bfloat16
F32 = mybir.dt.float32
AF = mybir.ActivationFunctionType
ALU = mybir.AluOpType
AX = mybir.AxisListType

V = 1000
NFEAT = 64
NHID = 64
NHEADS = 8
NCLASS = 10
POOL = 8
NN_EMBED = 64
EMBED = 256
ALPHA = 0.2
B = 32
NCORES = 8
BL = B // NCORES  # 4 batches per core
MASKC = 150.0

STOP_AFTER = "full"  # debug knob: "A" | "B" | "C" | "full"

VP = 1024  # padded node dim
VPT = VP // 128  # 8 node tiles
CHUNKS = ((0, 512), (512, 488))  # i-chunks aligned to PSUM banks


def build_kernel(nc, leaky_on_act_tiles=0):
    dt = {}

    def din(name, shape, dtype):
        dt[name] = nc.dram_tensor(name, list(shape), dtype, kind="ExternalInput").ap()

    def dout(name, shape, dtype):
        dt[name] = nc.dram_tensor(name, list(shape), dtype, kind="ExternalOutput").ap()

    din("xT", (VP, BL), F32)
    din("xrow", (BL, VP), F32)
    din("Lr", (VP, VP), F32)
    din("moff", (VP, VP), BF16)  # 150*(mask-1): 0 kept / -150 masked
    din("clwT", (3, NFEAT), F32)
    din("clb", (NFEAT, 1), F32)
    din("Wall", (NFEAT, NHEADS * NHID), BF16)
    din("qbig", (NFEAT + 1, 4 * NHEADS), F32)
    din("fc1wT", (500, 128, EMBED), BF16)
    din("fc1b", (128, 2), F32)
    din("decwT", (EMBED, VP), F32)
    din("decb", (128, VPT), F32)
    din("nn1wT", (VP, 512), F32)
    din("nn1b", (128, 4), F32)
    din("nn2wT", (512, NN_EMBED), F32)
    din("nn2b", (NN_EMBED, 1), F32)
    din("fc2wT", (EMBED + NN_EMBED, NCLASS), F32)
    din("fc2b", (1, NCLASS), F32)
    din("identbf", (128, 128), BF16)
    din("identf", (128, 128), F32)

    dout("xrecT", (VP, BL), F32)
    dout("ypred", (BL, NCLASS), F32)

    with tile.TileContext(nc) as tc:
        _body(nc, tc, dt, leaky_on_act_tiles)
    return dt


def _body(nc, tc, dt, leaky_on_act_tiles):
    from contextlib import ExitStack

    ctx = ExitStack()
    with ctx:
        const = ctx.enter_context(tc.tile_pool(name="const", bufs=1))
        work = ctx.enter_context(tc.tile_pool(name="work", bufs=2))
        psA = ctx.enter_context(tc.tile_pool(name="psA", bufs=3, space="PSUM"))
        psB = ctx.enter_context(tc.tile_pool(name="psB", bufs=2, space="PSUM"))
        psC = ctx.enter_context(tc.tile_pool(name="psC", bufs=1, space="PSUM"))
        fc1pool = ctx.enter_context(tc.tile_pool(name="fc1w", bufs=24))
        fstage = ctx.enter_context(tc.tile_pool(name="fstage", bufs=3))
        ppool = ctx.enter_context(tc.tile_pool(name="pp", bufs=2))

        # ---------------- persistent constants ----------------
        def load(pool, name, shape, dtype, rearr=None):
            t = pool.tile(shape, dtype, tag=name)
            ap = dt[name]
            if rearr:
                ap = ap.rearrange(rearr, p=128)
            nc.sync.dma_start(out=t, in_=ap)
            return t

        moff_sb = load(const, "moff", [128, VPT, VP], BF16, "(t p) j -> p t j")
        identb_sb = load(const, "identbf", [128, 128], BF16)
        identf_sb = load(const, "identf", [128, 128], F32)
        xT_sb = load(const, "xT", [128, VPT, BL], F32, "(t p) b -> p t b")
        clwT_sb = load(const, "clwT", [3, NFEAT], F32)
        clb_sb = load(const, "clb", [NFEAT, 1], F32)
        Wall_sb = load(const, "Wall", [NFEAT, NHEADS * NHID], BF16)
        qbig_sb = load(const, "qbig", [NFEAT + 1, 4 * NHEADS], F32)
        fc1b_sb = load(const, "fc1b", [128, 2], F32)
        decb_sb = load(const, "decb", [128, VPT], F32)
        nn1b_sb = load(const, "nn1b", [128, 4], F32)
        nn2wT_sb = load(const, "nn2wT", [128, 4, NN_EMBED], F32, "(t p) j -> p t j")
        nn2b_sb = load(const, "nn2b", [NN_EMBED, 1], F32)
        fc2wT_sb = const.tile([128, 3, NCLASS], F32, tag="fc2wT")
        nc.sync.dma_start(
            out=fc2wT_sb[:, 0:2, :],
            in_=dt["fc2wT"][0:256, :].rearrange("(t p) j -> p t j", p=128),
        )
        nc.sync.dma_start(out=fc2wT_sb[0:64, 2, :], in_=dt["fc2wT"][256:320, :])
        fc2bb_sb = const.tile([BL, NCLASS], F32, tag="fc2bb")
        nc.sync.dma_start(
            out=fc2bb_sb,
            in_=bass.AP(
                tensor=dt["fc2b"].tensor,
                offset=dt["fc2b"].offset,
                ap=[[0, BL], [1, NCLASS]],
            ),
        )

        f_big = []
        H_aug = []
        pooled = const.tile([128, 500 * BL], BF16, tag="pooled")

        # ---------------- Stages A+B (scoped pool) ----------------
        with tc.tile_pool(name="stageAB", bufs=1) as sab, tc.tile_pool(
            name="lrstream", bufs=2
        ) as lrp:
            x1_sb = sab.tile([128, VPT, BL], F32, tag="x1")
            x2_sb = sab.tile([128, VPT, BL], F32, tag="x2")
            Lr_re = dt["Lr"].rearrange("(t p) j -> p t j", p=128)
            for dst, src in ((x1_sb, xT_sb), (x2_sb, x1_sb)):
                ps = psA.tile([128, VPT * BL], F32, tag="ps1")
                for kt in range(VPT):
                    lrt = lrp.tile([128, VP], F32, tag="lrt")
                    nc.sync.dma_start(out=lrt, in_=Lr_re[:, kt, :])
                    for vt in range(VPT):
                        nc.tensor.matmul(
                            ps[:, vt * BL : vt * BL + BL],
                            lrt[:, vt * 128 : vt * 128 + 128],
                            src[:, kt, :],
                            start=(kt == 0 and vt == 0),
                            stop=(kt == VPT - 1 and vt == VPT - 1),
                        )
                psv = ps.rearrange("p (t b) -> p t b", b=BL)
                if dst is x1_sb:
                    nc.vector.tensor_copy(dst, psv)
                else:
                    for vt in range(VPT):
                        nc.vector.scalar_tensor_tensor(
                            out=dst[:, vt, :],
                            in0=psv[:, vt, :],
                            scalar=2.0,
                            in1=xT_sb[:, vt, :],
                            op0=ALU.mult,
                            op1=ALU.subtract,
                        )

            x1T_sb = sab.tile([BL, VP], F32, tag="x1T")
            x2T_sb = sab.tile([BL, VP], F32, tag="x2T")
            for dstT, src in ((x1T_sb, x1_sb), (x2T_sb, x2_sb)):
                tps = psB.tile([BL, VP], F32, tag="ps2")
                for vt in range(VPT):
                    nc.tensor.matmul(
                        tps[:, vt * 128 : vt * 128 + 128],
                        src[:, vt, :],
                        identf_sb,
                        is_transpose=True,
                        start=(vt % 4 == 0),
                        stop=(vt % 4 == 3),
                    )
                nc.vector.tensor_copy(dstT, tps)
            Xkb = []
            for b in range(BL):
                xk = sab.tile([3, VP], F32, tag=f"Xkb{b}")
                nc.sync.dma_start(out=xk[0:1, :], in_=dt["xrow"][b : b + 1, :])
                nc.sync.dma_start(out=xk[1:2, :], in_=x1T_sb[b : b + 1, :])
                nc.sync.dma_start(out=xk[2:3, :], in_=x2T_sb[b : b + 1, :])
                Xkb.append(xk)

            # hT = relu(cl_wT^T @ XkT + cl_b): [65, VP] fp32 (row 64 = ones)
            for b in range(BL):
                hps = psB.tile([NFEAT, VP], F32, tag="ps2")
                for cs, w in ((0, 512), (512, 512)):
                    nc.tensor.matmul(
                        hps[:, cs : cs + w],
                        clwT_sb,
                        Xkb[b][:, cs : cs + w],
                        start=True,
                        stop=True,
                    )
                h_f = sab.tile([NFEAT + 1, VP], F32, tag="hT", bufs=2)
                nc.scalar.activation(
                    h_f[0:NFEAT, :], hps, AF.Relu, bias=clb_sb, scale=1.0
                )
                nc.vector.memset(h_f[NFEAT : NFEAT + 1, :], 1.0)
                nc.vector.memset(h_f[0:NFEAT, V:VP], 0.0)
                h_b = sab.tile([NFEAT, VP], BF16, tag="hTb", bufs=2)
                nc.vector.tensor_copy(h_b, h_f[0:NFEAT, :])

                if STOP_AFTER == "A":
                    continue
                # f_big rows: 2h=f2_h, 2h+1=ones | 16+2h=ones, 16+2h+1=f1_h
                fps = psB.tile([4 * NHEADS, VP], F32, tag="ps2")
                for cs, w in ((0, 512), (512, 512)):
                    nc.tensor.matmul(
                        fps[:, cs : cs + w],
                        qbig_sb,
                        h_f[:, cs : cs + w],
                        start=True,
                        stop=True,
                    )
                fb = const.tile([4 * NHEADS, VP], BF16, tag=f"fbig{b}")
                nc.vector.tensor_copy(fb, fps)
                f_big.append(fb)

                Ha = const.tile([128, VPT, NHEADS, NHID + 1], BF16, tag=f"haug{b}")
                for vt in range(VPT):
                    Hps = psA.tile([128, NHEADS * NHID], F32, tag="ps1")
                    nc.tensor.matmul(
                        Hps,
                        h_b[:, vt * 128 : vt * 128 + 128],
                        Wall_sb,
                        start=True,
                        stop=True,
                    )
                    nc.vector.tensor_copy(
                        Ha[:, vt, :, 0:NHID],
                        Hps.rearrange("p (h c) -> p h c", h=NHEADS),
                    )
                    nc.vector.memset(Ha[:, vt, :, NHID : NHID + 1], 1.0)
                H_aug.append(Ha)

        # ---------------- Stage C: attention + interleaved fc1 ----------------
        fc1ps = psC.tile([128, 2 * BL], F32, tag="fc1ps")

        if STOP_AFTER in ("A", "B"):
            zz = work.tile([128, VPT, BL], F32, tag="zz")
            nc.vector.memset(zz, 0.0)
            nc.sync.dma_start(
                out=dt["xrecT"].rearrange("(t p) b -> p t b", p=128), in_=zz
            )
            zy = work.tile([BL, NCLASS], F32, tag="zy")
            nc.vector.memset(zy, 0.0)
            nc.sync.dma_start(out=dt["ypred"], in_=zy)
            return

        for hp in range(NHEADS // 2):
            for hin in range(2):
                h = 2 * hp + hin
                for b in range(BL):
                    fl = fstage.tile([2, VP], BF16, tag="fl")
                    fr = fstage.tile([2, VP], BF16, tag="fr")
                    nc.sync.dma_start(out=fl, in_=f_big[b][2 * h : 2 * h + 2, :])
                    nc.sync.dma_start(
                        out=fr, in_=f_big[b][16 + 2 * h : 16 + 2 * h + 2, :]
                    )
                    P = ppool.tile([128, VPT, 1000], BF16, tag="P")
                    for jt in range(VPT):
                        for cs, w in CHUNKS:
                            Eps = psA.tile([128, 512], F32, tag="ps1")
                            nc.tensor.matmul(
                                Eps[:, 0:w],
                                identb_sb,
                                moff_sb[:, jt, cs : cs + w],
                                start=True,
                                stop=False,
                            )
                            nc.tensor.matmul(
                                Eps[:, 0:w],
                                fl[:, jt * 128 : jt * 128 + 128],
                                fr[:, cs : cs + w],
                                start=False,
                                stop=True,
                            )
                            nc.vector._custom_dve(
                                LEAKY_OP,
                                out=P[:, jt, cs : cs + w],
                                in0=Eps[:, 0:w],
                                s0=ALPHA,
                            )
                    P2 = P.rearrange("p a b -> p (a b)")
                    nc.scalar.activation(P2, P2, AF.Exp)
                    UT = psB.tile([NHID + 1, 1000], F32, tag="ps2")
                    for jt in range(VPT):
                        for cs, w in CHUNKS:
                            nc.tensor.matmul(
                                UT[:, cs : cs + w],
                                H_aug[b][:, jt, h, :],
                                P[:, jt, cs : cs + w],
                                start=(jt == 0),
                                stop=(jt == VPT - 1),
                            )
                    srec = work.tile([1, 1000], F32, tag="srec")
                    nc.vector.reciprocal(srec, UT[NHID : NHID + 1, :])
                    sbc = work.tile([NHID, 1000], F32, tag="sbc")
                    nc.gpsimd.partition_broadcast(sbc, srec)
                    nrm = work.tile([NHID, 1000], BF16, tag="nrm")
                    nc.vector.scalar_tensor_tensor(
                        out=nrm,
                        in0=UT[0:NHID, :],
                        scalar=0.0,
                        in1=sbc,
                        op0=ALU.max,
                        op1=ALU.mult,
                    )
                    pout = pooled[64 * hin : 64 * hin + NHID, :].rearrange(
                        "p (blk r) -> p blk r", r=16
                    )[:, :, hp * BL + b]
                    nc.vector.tensor_reduce(
                        out=pout,
                        in_=nrm.rearrange("p (blk w) -> p blk w", w=POOL),
                        axis=AX.X,
                        op=ALU.max,
                    )
            # fc1 partial for k-tiles t with t % 4 == hp
            for blk in range(125):
                t = blk * 4 + hp
                wt = fc1pool.tile([128, EMBED], BF16, tag="fc1t")
                nc.sync.dma_start(out=wt, in_=dt["fc1wT"][t, :, :])
                for mt in range(2):
                    nc.tensor.matmul(
                        fc1ps[:, mt * BL : mt * BL + BL],
                        wt[:, mt * 128 : mt * 128 + 128],
                        pooled[:, t * BL : t * BL + BL],
                        start=(hp == 0 and blk == 0 and mt == 0),
                        stop=(hp == 3 and blk == 124 and mt == 1),
                    )

        # ---------------- Stage D: tail ----------------
        if STOP_AFTER == "C":
            zz = work.tile([128, VPT, BL], F32, tag="zz")
            nc.vector.tensor_copy(zz[:, 0, :], fc1ps[:, 0:BL])
            for vt in range(1, VPT):
                nc.vector.memset(zz[:, vt, :], 0.0)
            nc.sync.dma_start(
                out=dt["xrecT"].rearrange("(t p) b -> p t b", p=128), in_=zz
            )
            zy = work.tile([BL, NCLASS], F32, tag="zy")
            nc.vector.memset(zy, 0.0)
            nc.sync.dma_start(out=dt["ypred"], in_=zy)
            return

        with tc.tile_pool(name="stageD", bufs=1) as sd:
            decwT_sb = load(sd, "decwT", [128, 2, VP], F32, "(t p) j -> p t j")
            nn1wT_sb = load(sd, "nn1wT", [128, VPT, 512], F32, "(t p) j -> p t j")

            h1_sb = sd.tile([128, 2, BL], F32, tag="h1")
            for mt in range(2):
                nc.scalar.activation(
                    h1_sb[:, mt, :],
                    fc1ps[:, mt * BL : mt * BL + BL],
                    AF.Relu,
                    bias=fc1b_sb[:, mt : mt + 1],
                    scale=1.0,
                )

            xrec_sb = sd.tile([128, VPT, BL], F32, tag="xrec")
            for vt in range(VPT):
                dps = psA.tile([128, BL], F32, tag="ps1")
                for kt in range(2):
                    nc.tensor.matmul(
                        dps,
                        decwT_sb[:, kt, vt * 128 : vt * 128 + 128],
                        h1_sb[:, kt, :],
                        start=(kt == 0),
                        stop=(kt == 1),
                    )
                nc.scalar.activation(
                    xrec_sb[:, vt, :], dps, AF.Identity,
                    bias=decb_sb[:, vt : vt + 1], scale=1.0,
                )
            nc.sync.dma_start(
                out=dt["xrecT"].rearrange("(t p) b -> p t b", p=128), in_=xrec_sb
            )
            if STOP_AFTER == "D1":
                zy = work.tile([BL, NCLASS], F32, tag="zy")
                nc.vector.memset(zy, 0.0)
                nc.sync.dma_start(out=dt["ypred"], in_=zy)
                return

            z1_sb = sd.tile([128, 4, BL], F32, tag="z1")
            for mt in range(4):
                zps = psA.tile([128, BL], F32, tag="ps1")
                for kt in range(VPT):
                    nc.tensor.matmul(
                        zps,
                        nn1wT_sb[:, kt, mt * 128 : mt * 128 + 128],
                        xT_sb[:, kt, :],
                        start=(kt == 0),
                        stop=(kt == VPT - 1),
                    )
                nc.scalar.activation(
                    z1_sb[:, mt, :], zps, AF.Relu,
                    bias=nn1b_sb[:, mt : mt + 1], scale=1.0,
                )
            z2ps = psA.tile([NN_EMBED, BL], F32, tag="ps1")
            for kt in range(4):
                nc.tensor.matmul(
                    z2ps,
                    nn2wT_sb[:, kt, :],
                    z1_sb[:, kt, :],
                    start=(kt == 0),
                    stop=(kt == 3),
                )
            z2_sb = sd.tile([NN_EMBED, BL], F32, tag="z2")
            nc.scalar.activation(z2_sb, z2ps, AF.Relu, bias=nn2b_sb, scale=1.0)
            if STOP_AFTER == "D2":
                zy = work.tile([BL, NCLASS], F32, tag="zy")
                nc.vector.memset(zy, 0.0)
                nc.sync.dma_start(out=dt["ypred"], in_=zy)
                return

            ops = psA.tile([BL, NCLASS], F32, tag="ps1")
            nc.tensor.matmul(
                ops, h1_sb[:, 0, :], fc2wT_sb[:, 0, :], start=True, stop=False
            )
            nc.tensor.matmul(
                ops, h1_sb[:, 1, :], fc2wT_sb[:, 1, :], start=False, stop=False
            )
            nc.tensor.matmul(
                ops, z2_sb, fc2wT_sb[0:64, 2, :], start=False, stop=True
            )
            o2 = work.tile([BL, NCLASS], F32, tag="o2")
            nc.vector.tensor_tensor(out=o2, in0=ops, in1=fc2bb_sb, op=ALU.add)
            nc.vector.tensor_scalar_max(out=o2, in0=o2, scalar1=0.0)
            mm_ = work.tile([BL, 1], F32, tag="mm_")
            nc.vector.tensor_reduce(out=mm_, in_=o2, axis=AX.X, op=ALU.max)
            negm = work.tile([BL, 1], F32, tag="negm")
            nc.vector.tensor_scalar_mul(out=negm, in0=mm_, scalar1=-1.0)
            eo = work.tile([BL, NCLASS], F32, tag="eo")
            nc.scalar.activation(eo, o2, AF.Exp, bias=negm, scale=1.0)
            se = work.tile([BL, 1], F32, tag="se")
            nc.vector.tensor_reduce(out=se, in_=eo, axis=AX.X, op=ALU.add)
            lnse = work.tile([BL, 1], F32, tag="lnse")
            nc.scalar.activation(lnse, se, AF.Ln)
            cc = work.tile([BL, 1], F32, tag="cc")
            nc.vector.tensor_tensor(out=cc, in0=negm, in1=lnse, op=ALU.subtract)
            y_sb = work.tile([BL, NCLASS], F32, tag="ysb")
            nc.scalar.activation(y_sb, o2, AF.Identity, bias=cc, scale=1.0)
            nc.sync.dma_start(out=dt["ypred"], in_=y_sb)


# ---------------------------------------------------------------------------
# Host side
# ---------------------------------------------------------------------------

_BUILT = {}


def _get_built():
    if "nc" not in _BUILT:
        nc = bacc_mod.Bacc(
            trn_type="TRN2", target_bir_lowering=False, debug=False
        )
        build_kernel(nc)
        nc.compile()
        _BUILT["nc"] = nc
    return _BUILT["nc"]


def prep_host(x, adj, conv_degree, cl_w, cl_b, att_W, att_a, fc1_w, fc1_b,
              fc2_w, fc2_b, dec_w, dec_b, nn1_w, nn1_b, nn2_w, nn2_b):
    bf = ml_dtypes.bfloat16
    f32 = np.float32
    x = np.asarray(x, f32)
    adj = np.asarray(adj, f32)
    assert int(conv_degree) == 3

    d = adj.sum(axis=1)
    dinv = np.where(d > 0, 1.0 / np.sqrt(np.maximum(d, 1e-12)), 0.0).astype(f32)
    L = np.eye(V, dtype=f32) - dinv[:, None] * adj * dinv[None, :]
    lmax = float(np.linalg.eigvalsh(L.astype(np.float64))[-1])
    Lr = (2.0 * L / lmax - np.eye(V, dtype=f32)).astype(f32)

    Lp = np.zeros((VP, VP), f32)
    Lp[:V, :V] = Lr
    moff = np.full((VP, VP), -MASKC, f32)
    moff[:V, :V] = (adj > 0) * MASKC - MASKC

    xT = np.zeros((VP, B), f32)
    xT[:V, :] = x.T
    xrow = np.zeros((B, VP), f32)
    xrow[:, :V] = x

    cl_w = np.asarray(cl_w, f32)
    att_W = np.asarray(att_W, f32)
    att_a = np.asarray(att_a, f32)
    Wall = att_W.transpose(1, 0, 2).reshape(NFEAT, NHEADS * NHID)
    qbig = np.zeros((NFEAT + 1, 4 * NHEADS), f32)
    for h in range(NHEADS):
        q1 = att_W[h] @ att_a[h, :NHID]
        q2 = att_W[h] @ att_a[h, NHID:]
        qbig[:NFEAT, 2 * h] = q2          # lhs row 2h   = f2
        qbig[NFEAT, 2 * h + 1] = 1.0      # lhs row 2h+1 = ones
        qbig[NFEAT, 16 + 2 * h] = 1.0     # rhs row 2h   = ones
        qbig[:NFEAT, 16 + 2 * h + 1] = q1  # rhs row 2h+1 = f1

    fc1_w = np.asarray(fc1_w, f32)  # [256, 64000]
    fw = fc1_w.reshape(EMBED, 125, NHEADS, NHID)
    fw = fw.transpose(1, 2, 3, 0).reshape(125, 4, 2, NHID, EMBED)
    fc1wT = np.ascontiguousarray(fw.reshape(500, 128, EMBED)).astype(bf)
    fc1b = np.asarray(fc1_b, f32).reshape(2, 128).T.copy()

    dec_w = np.asarray(dec_w, f32)  # [1000, 256]
    decwT = np.zeros((EMBED, VP), f32)
    decwT[:, :V] = dec_w.T
    decb = np.zeros((VPT, 128), f32)
    decb.reshape(-1)[:V] = np.asarray(dec_b, f32)
    decb = decb.T.copy()  # [128, VPT]

    nn1_w = np.asarray(nn1_w, f32)  # [512, 1000]
    nn1wT = np.zeros((VP, 512), f32)
    nn1wT[:V, :] = nn1_w.T
    nn1b = np.asarray(nn1_b, f32).reshape(4, 128).T.copy()
    nn2wT = np.asarray(nn2_w, f32).T.copy()
    nn2b = np.asarray(nn2_b, f32).reshape(NN_EMBED, 1)
    fc2wT = np.asarray(fc2_w, f32).T.copy()
    fc2b = np.asarray(fc2_b, f32).reshape(1, NCLASS)

    common = dict(
        Lr=Lp,
        moff=moff.astype(bf),
        clwT=np.ascontiguousarray(cl_w.T),
        clb=np.asarray(cl_b, f32).reshape(NFEAT, 1),
        Wall=Wall.astype(bf),
        qbig=qbig,
        fc1wT=fc1wT,
        fc1b=fc1b,
        decwT=decwT,
        decb=decb,
        nn1wT=nn1wT,
        nn1b=nn1b,
        nn2wT=nn2wT,
        nn2b=nn2b,
        fc2wT=fc2wT,
        fc2b=fc2b,
        identbf=np.eye(128, dtype=bf),
        identf=np.eye(128, dtype=f32),
    )
    in_maps = []
    for c in range(NCORES):
        sl = slice(c * BL, (c + 1) * BL)
        m = dict(common)
        m["xT"] = np.ascontiguousarray(xT[:, sl])
        m["xrow"] = np.ascontiguousarray(xrow[sl, :])
        in_maps.append(m)
    return in_maps


def run(in_maps, **kw):
    nc = _get_built()
    return bass_utils.run_bass_kernel_spmd(
        nc, in_maps, core_ids=list(range(NCORES)), **kw
    )


def gather(results):
    xrec = np.zeros((B, V), np.float32)
    yp = np.zeros((B, NCLASS), np.float32)
    for c, r in enumerate(results):
        xrec[c * BL : (c + 1) * BL, :] = r["xrecT"][:V, :].T
        yp[c * BL : (c + 1) * BL, :] = r["ypred"]
    return xrec, yp


def kernel(**inputs):
    in_maps = prep_host(**inputs)
    res = run(in_maps)
    return gather(res.results)
